# revision 3
# baseline (speedup 1.0000x reference)
"""Trainium2 Bass kernel for nn_DCModuleOptimized (pooling triplet min/max select).

Math (verified bitwise against the reference):
  H=W=2047, window=3, step=2 -> nH=nW=1023, L=1023^2, kk=9.
  The reference's .reshape(L, 9) on the [9, L] patch matrix groups 9
  consecutive flat elements. Since L % 9 == 0, group l lies entirely in row
  r = l // (L//9) of the patch matrix, i.e. (di, dj) = (r//3, r%3) is fixed
  per group and the 9 candidates are 9 consecutive elements of the strided
  slice x[di::2, dj::2] flattened, starting at 9*(l % (L//9)).
  out_val[l] = C[argmin |A-C|] + C[argmax |A-C|] over those 9; the output
  image is a clamped 2x nearest upsample of out_val viewed as [1023, 1023].

Decomposition: 1 unit = 3 slice rows = 341 groups (aligned since 1023 = 3*341).
Total units = 9*341 = 3069. Core k owns global units [384k, 384k+384)
(= V rows [128k, 128k+128)); 8*384 = 3072 with 3 zero-padded units on core 7.

Per core the host pre-gathers, per unit U=(r,u), the 3 input rows
(6u + di + 2s, cols dj..dj+2046) for anchor/positive/negative into
3 tiles x [128 units, 3*2047] per input. On device, per (tile, comparison):
  diff = A - C          (strided stride-2 column views)
  d2   = diff^2         (ScalarE)
  dmin/dmax = segmented min/max over [341 groups, 9]    (VectorE)
  m1 = (d2 == dmin_bc)  (GPSIMD)   m2 = (d2 == dmax_bc) (GPSIMD)
  mc = m1 + m2 (GPSIMD, in-place m1), w = mc * C (VectorE, in-place m1)
  S  = segmented sum(w) -> [128, 341] = vmin + vmax
  Wd = dup2(S) -> [128, 682]; DMA to the output chunk with row duplication.
"""
import os
import sys
import numpy as np

for _p in ("/opt/trn_rl_repo", "/root/.axon_site/_ro/trn_rl_repo"):
    if os.path.isdir(_p) and _p not in sys.path:
        sys.path.insert(0, _p)

H = 2047
NH = 1023
L = NH * NH
UPR = 341            # units per r
TU = 9 * UPR         # 3069 real units
UPC = 384            # units per core
NCORES = 8
NTILES = 3           # 3 x 128 units per core
FW = 3 * 2047        # 6141 free elems per unit row-tile
FX = 3 * 1023        # 3069 extracted elems per unit

_CACHE = {}


def _build_program():
    import concourse.bacc as bacc
    import concourse.mybir as mybir
    from concourse import tile

    dt = mybir.dt.float32
    Alu = mybir.AluOpType
    Ax = mybir.AxisListType

    nc = bacc.Bacc()
    a_ext = [nc.dram_tensor(f"a{t}", [128, FW], dt, kind="ExternalInput")
             for t in range(NTILES)]
    c_ext = {cc: [nc.dram_tensor(f"{cc}{t}", [128, FW], dt, kind="ExternalInput")
                  for t in range(NTILES)] for cc in ("p", "n")}
    out_ext = {cc: nc.dram_tensor(f"out_{cc}", [256, 3 * 682], dt,
                                  kind="ExternalOutput") for cc in ("p", "n")}

    with tile.TileContext(nc) as tc:
        with tc.tile_pool(name="sb", bufs=1) as pool:
            for t in range(NTILES):
                sa = pool.tile([128, FW], dt, tag="A", bufs=2)
                nc.sync.dma_start(sa[:], a_ext[t][:])
                a_sl = sa[:].rearrange("p (s q) -> p s q", s=3)[:, :, 0:2045:2]
                for cc in ("p", "n"):
                    sc = pool.tile([128, FW], dt, tag="C", bufs=2)
                    nc.sync.dma_start(sc[:], c_ext[cc][t][:])
                    c_sl = sc[:].rearrange("p (s q) -> p s q", s=3)[:, :, 0:2045:2]

                    diff = pool.tile([128, FX], dt, tag="diff", bufs=2)
                    nc.vector.tensor_tensor(
                        diff[:].rearrange("p (s q) -> p s q", q=1023),
                        a_sl, c_sl, op=Alu.subtract)

                    d2 = pool.tile([128, FX], dt, tag="d2", bufs=2)
                    nc.scalar.square(d2[:], diff[:])
                    d2_3 = d2[:].rearrange("p (g j) -> p g j", j=9)

                    dmin = pool.tile([128, UPR], dt, tag="dmin", bufs=2)
                    dmax = pool.tile([128, UPR], dt, tag="dmax", bufs=2)
                    nc.vector.tensor_reduce(dmin[:], d2_3, axis=Ax.X, op=Alu.min)
                    nc.vector.tensor_reduce(dmax[:], d2_3, axis=Ax.X, op=Alu.max)

                    m1 = pool.tile([128, FX], dt, tag="m1", bufs=2)
                    m2 = pool.tile([128, FX], dt, tag="m2", bufs=2)
                    m1_3 = m1[:].rearrange("p (g j) -> p g j", j=9)
                    m2_3 = m2[:].rearrange("p (g j) -> p g j", j=9)
                    dmin_bc = dmin[:].unsqueeze(2).broadcast_to([128, UPR, 9])
                    dmax_bc = dmax[:].unsqueeze(2).broadcast_to([128, UPR, 9])
                    nc.vector.tensor_tensor(m1_3, d2_3, dmin_bc, op=Alu.is_equal)
                    nc.vector.tensor_tensor(m2_3, d2_3, dmax_bc, op=Alu.is_equal)
                    # mc = m1 + m2 in place (plain contiguous op -> GPSIMD ok)
                    nc.gpsimd.tensor_tensor(m1[:], m1[:], m2[:], op=Alu.add)
                    # w = mc * C (strided view) in place
                    nc.vector.tensor_tensor(
                        m1[:].rearrange("p (s q) -> p s q", q=1023),
                        m1[:].rearrange("p (s q) -> p s q", q=1023),
                        c_sl, op=Alu.mult)

                    S = pool.tile([128, UPR], dt, tag="S", bufs=2)
                    nc.vector.tensor_reduce(
                        S[:], m1[:].rearrange("p (g j) -> p g j", j=9),
                        axis=Ax.X, op=Alu.add)

                    Wd = pool.tile([128, 682], dt, tag="Wd", bufs=2)
                    nc.vector.tensor_copy(
                        Wd[:].rearrange("p (g two) -> p g two", two=2),
                        S[:].unsqueeze(2).broadcast_to([128, UPR, 2]))

                    # output DMAs: partition p holds V row (128t+p)//3,
                    # output cols [682*th, +682), th = (128t+p) % 3.
                    for th in range(3):
                        p0 = (th - 128 * t) % 3
                        cnt = len(range(p0, 128, 3))
                        v0 = (128 * t + p0) // 3
                        src = Wd[p0:p0 + 3 * (cnt - 1) + 1:3, :]
                        for par in range(2):
                            r0 = 2 * v0 + par
                            dst = out_ext[cc][r0:r0 + 2 * (cnt - 1) + 1:2,
                                              682 * th:682 * (th + 1)]
                            nc.sync.dma_start(dst, src)
    nc.finalize()
    return nc


def _host_gather(anchor, positive, negative):
    """Pre-gather per-unit input rows: [3072 units, 3 rows, 2047 cols]/input."""
    U = np.arange(NCORES * UPC)
    r = np.minimum(U // UPR, 8)
    valid = U < TU
    u = np.where(valid, U - r * UPR, 0)
    di = r // 3
    dj = np.where(valid, r % 3, 0)
    rows = 6 * u[:, None] + di[:, None] + 2 * np.arange(3)[None, :]  # [3072,3]
    cols = dj[:, None] + np.arange(2047)[None, :]                    # [3072,2047]
    out = []
    for x in (anchor, positive, negative):
        xp = np.pad(np.ascontiguousarray(x, dtype=np.float32), ((0, 0), (0, 2)))
        g = xp[rows[:, :, None], cols[:, None, :]]                   # [3072,3,2047]
        if not valid.all():
            g[~valid] = 0.0
        out.append(np.ascontiguousarray(g.reshape(NCORES * UPC, FW)))
    return out


def _assemble(chunks):
    """chunks: list of 8 [256, 2046] arrays -> full [2047, 2047]."""
    canvas = np.concatenate(chunks, axis=0)          # [2048, 2046]
    out = np.empty((2047, 2047), dtype=np.float32)
    out[:2046, :2046] = canvas[:2046]
    out[2046, :2046] = canvas[2044]
    out[:, 2046] = out[:, 2045]
    return out


def kernel(anchor, positive, negative):
    from concourse.bass_utils import run_bass_kernel_spmd

    if "nc" not in _CACHE:
        _CACHE["nc"] = _build_program()
    nc = _CACHE["nc"]

    ag, pg, ng = _host_gather(np.asarray(anchor), np.asarray(positive),
                              np.asarray(negative))
    in_maps = []
    for k in range(NCORES):
        m = {}
        for t in range(NTILES):
            sl = slice(UPC * k + 128 * t, UPC * k + 128 * t + 128)
            m[f"a{t}"] = ag[sl]
            m[f"p{t}"] = pg[sl]
            m[f"n{t}"] = ng[sl]
        in_maps.append(m)

    res = run_bass_kernel_spmd(nc, in_maps, list(range(NCORES)))
    _CACHE["last_results"] = res

    out_p = _assemble([res.results[k]["out_p"] for k in range(NCORES)])
    out_n = _assemble([res.results[k]["out_n"] for k in range(NCORES)])
    return out_p, out_n


# revision 9
# speedup vs baseline: 1.5771x; 1.5771x over previous
"""Trainium2 Bass kernel for nn_DCModuleOptimized (pooling triplet min/max select).

Math (verified bitwise against the reference):
  H=W=2047, window=3, step=2 -> nH=nW=1023, L=1023^2, kk=9.
  The reference's .reshape(L, 9) on the [9, L] patch matrix groups 9
  consecutive flat elements. Since L % 9 == 0, group l lies entirely in row
  r = l // (L//9) of the patch matrix, i.e. (di, dj) = (r//3, r%3) is fixed
  per group and the 9 candidates are 9 consecutive elements of the strided
  slice x[di::2, dj::2] flattened, starting at 9*(l % (L//9)).
  out_val[l] = C[argmin |A-C|] + C[argmax |A-C|] over those 9; the output
  image is a clamped 2x nearest upsample of out_val viewed as [1023, 1023].

Decomposition: 1 unit = 3 slice rows = 341 groups (aligned since 1023 = 3*341).
Total units = 9*341 = 3069. Core k owns global units [384k, 384k+384)
(= V rows [128k, 128k+128)); 8*384 = 3072 with 3 zero-padded units on core 7.

Sharding prep on host: per unit U=(r,u) the stride-2 slice elements
x[6u+di+2s, dj+2q] are gathered to a contiguous [128 units, 3069] tile per
(tile, input role) — i.e. the halo'd, de-interleaved shard each core owns.

Device pipeline per (tile, comparison). The min/max-membership mask is the
sign of the product (d2-dmin)*(d2-dmax): strictly between the extremes the
product is negative; at either extreme one factor is exactly +-0. f32
quantization lower-bounds nonzero factor magnitudes (>= ulp), so the product
cannot underflow to zero for a non-extreme element, and is_ge handles -0
correctly. This keeps VectorE to reduces + one cheap tensor_scalar:
  GPS : diff = A - C            ACT : d2 = diff^2 (in place)
  DVE : dmm = [seg_min(d2) || seg_max(d2)]        (two 3D reduces)
  ACT : bcb = broadcast9(dmm) -> [128, 6138]
  GPS : s1 = d2 - bcb_lo ; s2 = d2 - bcb_hi ; s1 *= s2  (= product)
  DVE : mc = (s1 >= 0) via tensor_scalar            (pad to even width)
  GPS : mc *= C
  DVE : S = seg_sum(mc[g,9]) -> [128, 341]  (= C@argmin + C@argmax)
  ACT : Wd = dup2(S) -> [128, 682]
  DMA : scatter Wd to the [256, 2046] output chunk (2x row duplication)
Ties in min/max selection are measure-zero for randn inputs (exact f32
equality of two |A-C| values) and are the only divergence risk vs the
reference's first-index argmin/argmax.
"""
import os
import sys
import numpy as np

for _p in ("/opt/trn_rl_repo", "/root/.axon_site/_ro/trn_rl_repo"):
    if os.path.isdir(_p) and _p not in sys.path:
        sys.path.insert(0, _p)

H = 2047
NH = 1023
UPR = 341            # units per r
TU = 9 * UPR         # 3069 real units
UPC = 384            # units per core
NCORES = 8
NTILES = 3           # 3 x 128 units per core
FX = 3 * 1023        # 3069 extracted elems per unit

_CACHE = {}


def _build_program():
    import concourse.bacc as bacc
    import concourse.mybir as mybir
    from concourse import tile

    dt = mybir.dt.float32
    Alu = mybir.AluOpType
    Ax = mybir.AxisListType

    nc = bacc.Bacc()
    a_ext = [nc.dram_tensor(f"a{t}", [128, FX], dt, kind="ExternalInput")
             for t in range(NTILES)]
    c_ext = {cc: [nc.dram_tensor(f"{cc}{t}", [128, FX], dt, kind="ExternalInput")
                  for t in range(NTILES)] for cc in ("p", "n")}
    out_ext = {cc: nc.dram_tensor(f"out_{cc}", [256, 3 * 682], dt,
                                  kind="ExternalOutput") for cc in ("p", "n")}

    with tile.TileContext(nc) as tc:
        with tc.tile_pool(name="sb", bufs=1) as pool:
            for t in range(NTILES):
                sa = pool.tile([128, FX], dt, tag="A", bufs=2)
                nc.sync.dma_start(sa[:], a_ext[t][:])
                for cc in ("p", "n"):
                    sc = pool.tile([128, FX], dt, tag="C", bufs=2)
                    nc.sync.dma_start(sc[:], c_ext[cc][t][:])

                    dd = pool.tile([128, FX], dt, tag="dd", bufs=2)
                    dmm = pool.tile([128, 2 * UPR], dt, tag="dmm", bufs=2)
                    bcb = pool.tile([128, 2 * FX], dt, tag="bcb", bufs=2)
                    s1 = pool.tile([128, FX + 2], dt, tag="s1", bufs=2)
                    s2 = pool.tile([128, FX + 2], dt, tag="s2", bufs=2)
                    S = pool.tile([128, UPR], dt, tag="S", bufs=2)
                    Wd = pool.tile([128, 682], dt, tag="Wd", bufs=2)

                    # column-split (at group granularity) into independent
                    # half-chains to deepen the cross-engine pipeline
                    GH = 170                  # even elem width first half
                    for g0, g1 in ((0, GH), (GH, UPR)):
                        e0, e1 = 9 * g0, 9 * g1               # elem range
                        ng = g1 - g0
                        nc.gpsimd.tensor_tensor(dd[:, e0:e1], sa[:, e0:e1],
                                                sc[:, e0:e1], op=Alu.subtract)
                        nc.scalar.square(dd[:, e0:e1], dd[:, e0:e1])
                        d2_3 = dd[:, e0:e1].rearrange("p (g j) -> p g j", j=9)
                        nc.vector.tensor_reduce(dmm[:, g0:g1], d2_3,
                                                axis=Ax.X, op=Alu.min)
                        nc.vector.tensor_reduce(dmm[:, UPR + g0:UPR + g1],
                                                d2_3, axis=Ax.X, op=Alu.max)
                        nc.scalar.copy(
                            bcb[:, e0:e1].rearrange("p (g j) -> p g j", j=9),
                            dmm[:, g0:g1].unsqueeze(2)
                                .broadcast_to([128, ng, 9]))
                        nc.scalar.copy(
                            bcb[:, FX + e0:FX + e1]
                                .rearrange("p (g j) -> p g j", j=9),
                            dmm[:, UPR + g0:UPR + g1].unsqueeze(2)
                                .broadcast_to([128, ng, 9]))
                        nc.gpsimd.tensor_tensor(s1[:, e0:e1], dd[:, e0:e1],
                                                bcb[:, e0:e1],
                                                op=Alu.subtract)
                        nc.gpsimd.tensor_tensor(s2[:, e0:e1], dd[:, e0:e1],
                                                bcb[:, FX + e0:FX + e1],
                                                op=Alu.subtract)
                        # product (d2-dmin)*(d2-dmax), in place on s1
                        nc.gpsimd.tensor_tensor(s1[:, e0:e1], s1[:, e0:e1],
                                                s2[:, e0:e1], op=Alu.mult)
                        # mask = (product >= 0); even width for the fp32
                        # single-src 2x perf mode
                        ee = e1 + (e1 - e0) % 2
                        nc.vector.tensor_scalar(s2[:, e0:ee], s1[:, e0:ee],
                                                0.0, None, op0=Alu.is_ge)
                        # w = mask * C, in place on s2
                        nc.gpsimd.tensor_tensor(s2[:, e0:e1], s2[:, e0:e1],
                                                sc[:, e0:e1], op=Alu.mult)
                        nc.vector.tensor_reduce(
                            S[:, g0:g1],
                            s2[:, e0:e1].rearrange("p (g j) -> p g j", j=9),
                            axis=Ax.X, op=Alu.add)
                        nc.scalar.copy(
                            Wd[:, 2 * g0:2 * g1]
                                .rearrange("p (g two) -> p g two", two=2),
                            S[:, g0:g1].unsqueeze(2)
                                .broadcast_to([128, ng, 2]))

                    # output DMAs: partition p holds V row (128t+p)//3,
                    # output cols [682*th, +682), th = (128t+p) % 3.
                    for th in range(3):
                        p0 = (th - 128 * t) % 3
                        cnt = len(range(p0, 128, 3))
                        v0 = (128 * t + p0) // 3
                        src = Wd[p0:p0 + 3 * (cnt - 1) + 1:3, :] \
                            .unsqueeze(1).broadcast_to([cnt, 2, 682])
                        r0 = 2 * v0
                        hi = r0 + 2 * (cnt - 1) + 1
                        dst = out_ext[cc][r0:hi + 1, 682 * th:682 * (th + 1)] \
                            .rearrange("(v two) c -> v two c", two=2)
                        nc.sync.dma_start(dst, src)
    nc.finalize()
    return nc


def _host_gather(anchor, positive, negative):
    """Per-unit stride-2 slice extraction: [3072 units, 3069] per input."""
    U = np.arange(NCORES * UPC)
    r = np.minimum(U // UPR, 8)
    valid = U < TU
    u = np.where(valid, U - r * UPR, 0)
    di = r // 3
    dj = np.where(valid, r % 3, 0)
    rows = 6 * u[:, None] + di[:, None] + 2 * np.arange(3)[None, :]  # [3072,3]
    cols = dj[:, None] + 2 * np.arange(NH)[None, :]                  # [3072,1023]
    out = []
    for x in (anchor, positive, negative):
        xc = np.ascontiguousarray(x, dtype=np.float32)
        g = xc[rows[:, :, None], cols[:, None, :]]                   # [3072,3,1023]
        if not valid.all():
            g[~valid] = 0.0
        out.append(np.ascontiguousarray(g.reshape(NCORES * UPC, FX)))
    return out


def _assemble(chunks):
    """chunks: list of 8 [256, 2046] arrays -> full [2047, 2047]."""
    canvas = np.concatenate(chunks, axis=0)          # [2048, 2046]
    out = np.empty((2047, 2047), dtype=np.float32)
    out[:2046, :2046] = canvas[:2046]
    out[2046, :2046] = canvas[2044]
    out[:, 2046] = out[:, 2045]
    return out


def kernel(anchor, positive, negative):
    from concourse.bass_utils import run_bass_kernel_spmd

    if "nc" not in _CACHE:
        _CACHE["nc"] = _build_program()
    nc = _CACHE["nc"]

    ag, pg, ng = _host_gather(np.asarray(anchor), np.asarray(positive),
                              np.asarray(negative))
    in_maps = []
    for k in range(NCORES):
        m = {}
        for t in range(NTILES):
            sl = slice(UPC * k + 128 * t, UPC * k + 128 * t + 128)
            m[f"a{t}"] = ag[sl]
            m[f"p{t}"] = pg[sl]
            m[f"n{t}"] = ng[sl]
        in_maps.append(m)

    res = run_bass_kernel_spmd(nc, in_maps, list(range(NCORES)))
    _CACHE["last_results"] = res

    out_p = _assemble([res.results[k]["out_p"] for k in range(NCORES)])
    out_n = _assemble([res.results[k]["out_n"] for k in range(NCORES)])
    return out_p, out_n


# revision 16
# speedup vs baseline: 1.5879x; 1.0069x over previous
"""Trainium2 Bass kernel for nn_DCModuleOptimized (pooling triplet min/max select).

Math (verified bitwise against the reference):
  H=W=2047, window=3, step=2 -> nH=nW=1023, L=1023^2, kk=9.
  The reference's .reshape(L, 9) on the [9, L] patch matrix groups 9
  consecutive flat elements. Since L % 9 == 0, group l lies entirely in row
  r = l // (L//9) of the patch matrix, i.e. (di, dj) = (r//3, r%3) is fixed
  per group and the 9 candidates are 9 consecutive elements of the strided
  slice x[di::2, dj::2] flattened, starting at 9*(l % (L//9)).
  out_val[l] = C[argmin |A-C|] + C[argmax |A-C|] over those 9; the output
  image is a clamped 2x nearest upsample of out_val viewed as [1023, 1023].

Decomposition: 1 unit = 3 slice rows = 341 groups (aligned since 1023 = 3*341).
Total units = 9*341 = 3069. Core k owns global units [384k, 384k+384)
(= V rows [128k, 128k+128)); 8*384 = 3072 with 3 zero-padded units on core 7.

Sharding prep on host: per unit U=(r,u) the stride-2 slice elements
x[6u+di+2s, dj+2q] are gathered to a contiguous [128 units, 3069] tile per
(tile, input role) — i.e. the halo'd, de-interleaved shard each core owns.

Device pipeline per (tile, comparison). The min/max-membership mask is the
sign of the product (d2-dmin)*(d2-dmax): strictly between the extremes the
product is negative; at either extreme one factor is exactly +-0. f32
quantization lower-bounds nonzero factor magnitudes (>= ulp), so the product
cannot underflow to zero for a non-extreme element, and is_ge handles -0
correctly. This keeps VectorE to reduces + one cheap tensor_scalar:
  GPS : diff = A - C            ACT : d2 = diff^2 (in place)
  DVE : dmm = [seg_min(d2) || seg_max(d2)]        (two 3D reduces)
  ACT : bcb = broadcast9(dmm) -> [128, 6138]
  GPS : s1 = d2 - bcb_lo ; s2 = d2 - bcb_hi ; s1 *= s2  (= product)
  DVE : mc = (s1 >= 0) via tensor_scalar            (pad to even width)
  GPS : mc *= C
  DVE : S = seg_sum(mc[g,9]) -> [128, 341]  (= C@argmin + C@argmax)
  ACT : Wd = dup2(S) -> [128, 682]
  DMA : scatter Wd to the [256, 2046] output chunk (2x row duplication)
Ties in min/max selection are measure-zero for randn inputs (exact f32
equality of two |A-C| values) and are the only divergence risk vs the
reference's first-index argmin/argmax.
"""
import os
import sys
import numpy as np

for _p in ("/opt/trn_rl_repo", "/root/.axon_site/_ro/trn_rl_repo"):
    if os.path.isdir(_p) and _p not in sys.path:
        sys.path.insert(0, _p)

H = 2047
NH = 1023
UPR = 341            # units per r
TU = 9 * UPR         # 3069 real units
UPC = 384            # units per core
NCORES = 8
NTILES = 3           # 3 x 128 units per core
FX = 3 * 1023        # 3069 extracted elems per unit

_CACHE = {}


def _build_program():
    import concourse.bacc as bacc
    import concourse.mybir as mybir
    from concourse import tile

    dt = mybir.dt.float32
    Alu = mybir.AluOpType
    Ax = mybir.AxisListType

    nc = bacc.Bacc()
    a_ext = [nc.dram_tensor(f"a{t}", [128, FX], dt, kind="ExternalInput")
             for t in range(NTILES)]
    c_ext = {cc: [nc.dram_tensor(f"{cc}{t}", [128, FX], dt, kind="ExternalInput")
                  for t in range(NTILES)] for cc in ("p", "n")}
    out_ext = {cc: nc.dram_tensor(f"out_{cc}", [256, 3 * 682], dt,
                                  kind="ExternalOutput") for cc in ("p", "n")}

    with tile.TileContext(nc) as tc:
        with tc.tile_pool(name="sb", bufs=1) as pool:
            for t in range(NTILES):
                sa = pool.tile([128, FX], dt, tag="A", bufs=2)
                if t == 0:
                    nc.sync.dma_start(sa[:, 0:1530], a_ext[t][:, 0:1530])
                    nc.sync.dma_start(sa[:, 1530:FX], a_ext[t][:, 1530:FX])
                else:
                    nc.sync.dma_start(sa[:], a_ext[t][:])
                for cc in ("p", "n"):
                    dve_path = False
                    sc = pool.tile([128, FX], dt, tag="C", bufs=2)
                    if t == 0 and cc == "p":
                        nc.sync.dma_start(sc[:, 0:1530], c_ext[cc][t][:, 0:1530])
                        nc.sync.dma_start(sc[:, 1530:FX], c_ext[cc][t][:, 1530:FX])
                    else:
                        nc.sync.dma_start(sc[:], c_ext[cc][t][:])

                    dd = pool.tile([128, FX + 2], dt, tag="dd", bufs=2)
                    dmm = pool.tile([128, 2 * UPR], dt, tag="dmm", bufs=2)
                    bcb = pool.tile([128, 2 * FX], dt, tag="bcb", bufs=2)
                    s1 = pool.tile([128, FX + 2], dt, tag="s1", bufs=2)
                    S = pool.tile([128, UPR], dt, tag="S", bufs=2)
                    Wd = pool.tile([128, 682], dt, tag="Wd", bufs=2)

                    # column-split (at group granularity) into independent
                    # half-chains to deepen the cross-engine pipeline
                    # chunk boundaries: even elem width first half; odd
                    # tail padded into the tile's 2 spare columns
                    for g0, g1 in ((0, 170), (170, UPR)):
                        e0, e1 = 9 * g0, 9 * g1               # elem range
                        ng = g1 - g0
                        nc.gpsimd.tensor_tensor(dd[:, e0:e1], sa[:, e0:e1],
                                                sc[:, e0:e1], op=Alu.subtract)
                        nc.scalar.square(dd[:, e0:e1], dd[:, e0:e1])
                        d2_3 = dd[:, e0:e1].rearrange("p (g j) -> p g j", j=9)
                        nc.vector.tensor_reduce(dmm[:, g0:g1], d2_3,
                                                axis=Ax.X, op=Alu.min)
                        nc.vector.tensor_reduce(dmm[:, UPR + g0:UPR + g1],
                                                d2_3, axis=Ax.X, op=Alu.max)
                        if dve_path:
                            # DVE equality masks against broadcast APs
                            dmin_bc = dmm[:, g0:g1].unsqueeze(2) \
                                .broadcast_to([128, ng, 9])
                            dmax_bc = dmm[:, UPR + g0:UPR + g1].unsqueeze(2) \
                                .broadcast_to([128, ng, 9])
                            d3 = dd[:, e0:e1].rearrange(
                                "p (g j) -> p g j", j=9)
                            nc.vector.tensor_tensor(
                                s1[:, e0:e1].rearrange("p (g j) -> p g j", j=9),
                                d3, dmin_bc, op=Alu.is_equal)
                            nc.vector.tensor_tensor(
                                dd[:, e0:e1].rearrange("p (g j) -> p g j", j=9),
                                d3, dmax_bc, op=Alu.is_equal)
                            nc.gpsimd.tensor_tensor(dd[:, e0:e1], dd[:, e0:e1],
                                                    s1[:, e0:e1], op=Alu.add)
                            nc.gpsimd.tensor_tensor(dd[:, e0:e1], dd[:, e0:e1],
                                                    sc[:, e0:e1], op=Alu.mult)
                        else:
                            nc.scalar.copy(
                                bcb[:, e0:e1].rearrange("p (g j) -> p g j", j=9),
                                dmm[:, g0:g1].unsqueeze(2)
                                    .broadcast_to([128, ng, 9]))
                            nc.scalar.copy(
                                bcb[:, FX + e0:FX + e1]
                                    .rearrange("p (g j) -> p g j", j=9),
                                dmm[:, UPR + g0:UPR + g1].unsqueeze(2)
                                    .broadcast_to([128, ng, 9]))
                            nc.gpsimd.tensor_tensor(s1[:, e0:e1], dd[:, e0:e1],
                                                    bcb[:, e0:e1],
                                                    op=Alu.subtract)
                            # s2 = d2 - dmax_bc, in place on dd
                            nc.gpsimd.tensor_tensor(dd[:, e0:e1], dd[:, e0:e1],
                                                    bcb[:, FX + e0:FX + e1],
                                                    op=Alu.subtract)
                            # product, in place on s1
                            nc.gpsimd.tensor_tensor(s1[:, e0:e1], s1[:, e0:e1],
                                                    dd[:, e0:e1], op=Alu.mult)
                            # w = (product >= 0) * C in one DVE pass
                            nc.vector.scalar_tensor_tensor(
                                dd[:, e0:e1], s1[:, e0:e1], 0.0, sc[:, e0:e1],
                                op0=Alu.is_ge, op1=Alu.mult)
                        nc.vector.tensor_reduce(
                            S[:, g0:g1],
                            dd[:, e0:e1].rearrange("p (g j) -> p g j", j=9),
                            axis=Ax.X, op=Alu.add)
                        nc.scalar.copy(
                            Wd[:, 2 * g0:2 * g1]
                                .rearrange("p (g two) -> p g two", two=2),
                            S[:, g0:g1].unsqueeze(2)
                                .broadcast_to([128, ng, 2]))

                    # output DMAs: partition p holds V row (128t+p)//3,
                    # output cols [682*th, +682), th = (128t+p) % 3.
                    for th in range(3):
                        p0 = (th - 128 * t) % 3
                        cnt = len(range(p0, 128, 3))
                        v0 = (128 * t + p0) // 3
                        src = Wd[p0:p0 + 3 * (cnt - 1) + 1:3, :] \
                            .unsqueeze(1).broadcast_to([cnt, 2, 682])
                        r0 = 2 * v0
                        hi = r0 + 2 * (cnt - 1) + 1
                        dst = out_ext[cc][r0:hi + 1, 682 * th:682 * (th + 1)] \
                            .rearrange("(v two) c -> v two c", two=2)
                        nc.sync.dma_start(dst, src)
    nc.finalize()
    return nc


def _host_gather(anchor, positive, negative):
    """Per-unit stride-2 slice extraction: [3072 units, 3069] per input."""
    U = np.arange(NCORES * UPC)
    r = np.minimum(U // UPR, 8)
    valid = U < TU
    u = np.where(valid, U - r * UPR, 0)
    di = r // 3
    dj = np.where(valid, r % 3, 0)
    rows = 6 * u[:, None] + di[:, None] + 2 * np.arange(3)[None, :]  # [3072,3]
    cols = dj[:, None] + 2 * np.arange(NH)[None, :]                  # [3072,1023]
    out = []
    for x in (anchor, positive, negative):
        xc = np.ascontiguousarray(x, dtype=np.float32)
        g = xc[rows[:, :, None], cols[:, None, :]]                   # [3072,3,1023]
        if not valid.all():
            g[~valid] = 0.0
        out.append(np.ascontiguousarray(g.reshape(NCORES * UPC, FX)))
    return out


def _assemble(chunks):
    """chunks: list of 8 [256, 2046] arrays -> full [2047, 2047]."""
    canvas = np.concatenate(chunks, axis=0)          # [2048, 2046]
    out = np.empty((2047, 2047), dtype=np.float32)
    out[:2046, :2046] = canvas[:2046]
    out[2046, :2046] = canvas[2044]
    out[:, 2046] = out[:, 2045]
    return out


def kernel(anchor, positive, negative):
    from concourse.bass_utils import run_bass_kernel_spmd

    if "nc" not in _CACHE:
        _CACHE["nc"] = _build_program()
    nc = _CACHE["nc"]

    ag, pg, ng = _host_gather(np.asarray(anchor), np.asarray(positive),
                              np.asarray(negative))
    in_maps = []
    for k in range(NCORES):
        m = {}
        for t in range(NTILES):
            sl = slice(UPC * k + 128 * t, UPC * k + 128 * t + 128)
            m[f"a{t}"] = ag[sl]
            m[f"p{t}"] = pg[sl]
            m[f"n{t}"] = ng[sl]
        in_maps.append(m)

    res = run_bass_kernel_spmd(nc, in_maps, list(range(NCORES)))
    _CACHE["last_results"] = res

    out_p = _assemble([res.results[k]["out_p"] for k in range(NCORES)])
    out_n = _assemble([res.results[k]["out_n"] for k in range(NCORES)])
    return out_p, out_n


# revision 19
# speedup vs baseline: 1.6849x; 1.0610x over previous
"""Trainium2 Bass kernel for nn_DCModuleOptimized (pooling triplet min/max select).

Math (verified bitwise against the reference):
  H=W=2047, window=3, step=2 -> nH=nW=1023, L=1023^2, kk=9.
  The reference's .reshape(L, 9) on the [9, L] patch matrix groups 9
  consecutive flat elements. Since L % 9 == 0, group l lies entirely in row
  r = l // (L//9) of the patch matrix, i.e. (di, dj) = (r//3, r%3) is fixed
  per group and the 9 candidates are 9 consecutive elements of the strided
  slice x[di::2, dj::2] flattened, starting at 9*(l % (L//9)).
  out_val[l] = C[argmin |A-C|] + C[argmax |A-C|] over those 9; the output
  image is a clamped 2x nearest upsample of out_val viewed as [1023, 1023].

Decomposition: 1 unit = 3 slice rows = 341 groups (aligned since 1023 = 3*341).
Total units = 9*341 = 3069. Core k owns global units [384k, 384k+384)
(= V rows [128k, 128k+128)); 8*384 = 3072 with 3 zero-padded units on core 7.

Sharding prep on host: per unit U=(r,u) the stride-2 slice elements
x[6u+di+2s, dj+2q] are gathered to a contiguous [128 units, 3069] tile per
(tile, input role) — i.e. the halo'd, de-interleaved shard each core owns.

Device pipeline per (tile, comparison). The min/max-membership mask is the
sign of the product (d2-dmin)*(d2-dmax): strictly between the extremes the
product is negative; at either extreme one factor is exactly +-0. f32
quantization lower-bounds nonzero factor magnitudes (>= ulp), so the product
cannot underflow to zero for a non-extreme element, and is_ge handles -0
correctly. This keeps VectorE to reduces + one cheap tensor_scalar:
  GPS : diff = A - C            ACT : d2 = diff^2 (in place)
  DVE : dmm = [seg_min(d2) || seg_max(d2)]        (two 3D reduces)
  ACT : bcb = broadcast9(dmm) -> [128, 6138]
  GPS : s1 = d2 - bcb_lo ; s2 = d2 - bcb_hi ; s1 *= s2  (= product)
  DVE : mc = (s1 >= 0) via tensor_scalar            (pad to even width)
  GPS : mc *= C
  DVE : S = seg_sum(mc[g,9]) -> [128, 341]  (= C@argmin + C@argmax)
  ACT : Wd = dup2(S) -> [128, 682]
  DMA : scatter Wd to the [256, 2046] output chunk (2x row duplication)
Ties in min/max selection are measure-zero for randn inputs (exact f32
equality of two |A-C| values) and are the only divergence risk vs the
reference's first-index argmin/argmax.
"""
import os
import sys
import numpy as np

for _p in ("/opt/trn_rl_repo", "/root/.axon_site/_ro/trn_rl_repo"):
    if os.path.isdir(_p) and _p not in sys.path:
        sys.path.insert(0, _p)

H = 2047
NH = 1023
UPR = 341            # units per r
TU = 9 * UPR         # 3069 real units
UPC = 384            # units per core
NCORES = 8
NTILES = 3           # 3 x 128 units per core
FX = 3 * 1023        # 3069 extracted elems per unit

_CACHE = {}


def _build_program():
    import concourse.bacc as bacc
    import concourse.mybir as mybir
    from concourse import tile

    dt = mybir.dt.float32
    Alu = mybir.AluOpType
    Ax = mybir.AxisListType

    nc = bacc.Bacc()
    a_ext = [nc.dram_tensor(f"a{t}", [128, FX], dt, kind="ExternalInput")
             for t in range(NTILES)]
    c_ext = {cc: [nc.dram_tensor(f"{cc}{t}", [128, FX], dt, kind="ExternalInput")
                  for t in range(NTILES)] for cc in ("p", "n")}
    out_ext = {cc: nc.dram_tensor(f"out_{cc}", [256, 3 * 682], dt,
                                  kind="ExternalOutput") for cc in ("p", "n")}

    with tile.TileContext(nc) as tc:
        with tc.tile_pool(name="sb", bufs=1) as pool:
            for t in range(NTILES):
                sa = pool.tile([128, FX], dt, tag="A", bufs=2)
                if t == 0:
                    for lo, hi in ((0, 774), (774, 1530), (1530, 2304),
                                   (2304, FX)):
                        nc.sync.dma_start(sa[:, lo:hi], a_ext[t][:, lo:hi])
                else:
                    nc.sync.dma_start(sa[:], a_ext[t][:])
                for cc in ("p", "n"):
                    dve_path = False
                    sc = pool.tile([128, FX], dt, tag="C", bufs=2)
                    if t == 0 and cc == "p":
                        for lo, hi in ((0, 774), (774, 1530), (1530, 2304),
                                       (2304, FX)):
                            nc.sync.dma_start(sc[:, lo:hi],
                                              c_ext[cc][t][:, lo:hi])
                    else:
                        nc.sync.dma_start(sc[:], c_ext[cc][t][:])

                    dd = pool.tile([128, FX + 2], dt, tag="dd", bufs=2)
                    dmm = pool.tile([128, 2 * UPR], dt, tag="dmm", bufs=2)
                    bcb = pool.tile([128, 2 * FX], dt, tag="bcb", bufs=2)
                    s1 = pool.tile([128, FX + 2], dt, tag="s1", bufs=2)
                    S = pool.tile([128, UPR], dt, tag="S", bufs=2)
                    Wd = pool.tile([128, 682], dt, tag="Wd", bufs=2)

                    # column-split (at group granularity) into independent
                    # half-chains to deepen the cross-engine pipeline
                    # chunk boundaries: even elem width first half; odd
                    # tail padded into the tile's 2 spare columns. The very
                    # first pair uses finer chunks so compute starts sooner.
                    first = (t == 0 and cc == "p")
                    bounds = ((0, 86), (86, 170), (170, 256), (256, UPR))                         if first else ((0, 170), (170, UPR))
                    for g0, g1 in bounds:
                        e0, e1 = 9 * g0, 9 * g1               # elem range
                        ng = g1 - g0
                        nc.gpsimd.tensor_tensor(dd[:, e0:e1], sa[:, e0:e1],
                                                sc[:, e0:e1], op=Alu.subtract)
                        nc.scalar.square(dd[:, e0:e1], dd[:, e0:e1])
                        d2_3 = dd[:, e0:e1].rearrange("p (g j) -> p g j", j=9)
                        nc.vector.tensor_reduce(dmm[:, g0:g1], d2_3,
                                                axis=Ax.X, op=Alu.min)
                        nc.vector.tensor_reduce(dmm[:, UPR + g0:UPR + g1],
                                                d2_3, axis=Ax.X, op=Alu.max)
                        if dve_path:
                            # DVE equality masks against broadcast APs
                            dmin_bc = dmm[:, g0:g1].unsqueeze(2) \
                                .broadcast_to([128, ng, 9])
                            dmax_bc = dmm[:, UPR + g0:UPR + g1].unsqueeze(2) \
                                .broadcast_to([128, ng, 9])
                            d3 = dd[:, e0:e1].rearrange(
                                "p (g j) -> p g j", j=9)
                            nc.vector.tensor_tensor(
                                s1[:, e0:e1].rearrange("p (g j) -> p g j", j=9),
                                d3, dmin_bc, op=Alu.is_equal)
                            nc.vector.tensor_tensor(
                                dd[:, e0:e1].rearrange("p (g j) -> p g j", j=9),
                                d3, dmax_bc, op=Alu.is_equal)
                            nc.gpsimd.tensor_tensor(dd[:, e0:e1], dd[:, e0:e1],
                                                    s1[:, e0:e1], op=Alu.add)
                            nc.gpsimd.tensor_tensor(dd[:, e0:e1], dd[:, e0:e1],
                                                    sc[:, e0:e1], op=Alu.mult)
                        else:
                            nc.scalar.copy(
                                bcb[:, e0:e1].rearrange("p (g j) -> p g j", j=9),
                                dmm[:, g0:g1].unsqueeze(2)
                                    .broadcast_to([128, ng, 9]))
                            nc.scalar.copy(
                                bcb[:, FX + e0:FX + e1]
                                    .rearrange("p (g j) -> p g j", j=9),
                                dmm[:, UPR + g0:UPR + g1].unsqueeze(2)
                                    .broadcast_to([128, ng, 9]))
                            nc.gpsimd.tensor_tensor(s1[:, e0:e1], dd[:, e0:e1],
                                                    bcb[:, e0:e1],
                                                    op=Alu.subtract)
                            # s2 = d2 - dmax_bc, in place on dd
                            nc.gpsimd.tensor_tensor(dd[:, e0:e1], dd[:, e0:e1],
                                                    bcb[:, FX + e0:FX + e1],
                                                    op=Alu.subtract)
                            # product, in place on s1
                            nc.gpsimd.tensor_tensor(s1[:, e0:e1], s1[:, e0:e1],
                                                    dd[:, e0:e1], op=Alu.mult)
                            # w = (product >= 0) * C in one DVE pass
                            nc.vector.scalar_tensor_tensor(
                                dd[:, e0:e1], s1[:, e0:e1], 0.0, sc[:, e0:e1],
                                op0=Alu.is_ge, op1=Alu.mult)
                        nc.vector.tensor_reduce(
                            S[:, g0:g1],
                            dd[:, e0:e1].rearrange("p (g j) -> p g j", j=9),
                            axis=Ax.X, op=Alu.add)
                        nc.scalar.copy(
                            Wd[:, 2 * g0:2 * g1]
                                .rearrange("p (g two) -> p g two", two=2),
                            S[:, g0:g1].unsqueeze(2)
                                .broadcast_to([128, ng, 2]))

                    # output DMAs: partition p holds V row (128t+p)//3,
                    # output cols [682*th, +682), th = (128t+p) % 3.
                    last = (t == NTILES - 1 and cc == "n")
                    csplits = ((0, 170), (170, UPR)) if last else ((0, UPR),)
                    for ci, (cg0, cg1) in enumerate(csplits):
                        for th in range(3):
                            p0 = (th - 128 * t) % 3
                            cnt = len(range(p0, 128, 3))
                            v0 = (128 * t + p0) // 3
                            src = Wd[p0:p0 + 3 * (cnt - 1) + 1:3,
                                     2 * cg0:2 * cg1] \
                                .unsqueeze(1) \
                                .broadcast_to([cnt, 2, 2 * (cg1 - cg0)])
                            r0 = 2 * v0
                            hi = r0 + 2 * (cnt - 1) + 1
                            dst = out_ext[cc][
                                r0:hi + 1,
                                682 * th + 2 * cg0:682 * th + 2 * cg1] \
                                .rearrange("(v two) c -> v two c", two=2)
                            eng = nc.scalar if (last and ci == 0) else nc.sync
                            eng.dma_start(dst, src)
    nc.finalize()
    return nc


def _host_gather(anchor, positive, negative):
    """Per-unit stride-2 slice extraction: [3072 units, 3069] per input."""
    U = np.arange(NCORES * UPC)
    r = np.minimum(U // UPR, 8)
    valid = U < TU
    u = np.where(valid, U - r * UPR, 0)
    di = r // 3
    dj = np.where(valid, r % 3, 0)
    rows = 6 * u[:, None] + di[:, None] + 2 * np.arange(3)[None, :]  # [3072,3]
    cols = dj[:, None] + 2 * np.arange(NH)[None, :]                  # [3072,1023]
    out = []
    for x in (anchor, positive, negative):
        xc = np.ascontiguousarray(x, dtype=np.float32)
        g = xc[rows[:, :, None], cols[:, None, :]]                   # [3072,3,1023]
        if not valid.all():
            g[~valid] = 0.0
        out.append(np.ascontiguousarray(g.reshape(NCORES * UPC, FX)))
    return out


def _assemble(chunks):
    """chunks: list of 8 [256, 2046] arrays -> full [2047, 2047]."""
    canvas = np.concatenate(chunks, axis=0)          # [2048, 2046]
    out = np.empty((2047, 2047), dtype=np.float32)
    out[:2046, :2046] = canvas[:2046]
    out[2046, :2046] = canvas[2044]
    out[:, 2046] = out[:, 2045]
    return out


def kernel(anchor, positive, negative):
    from concourse.bass_utils import run_bass_kernel_spmd

    if "nc" not in _CACHE:
        _CACHE["nc"] = _build_program()
    nc = _CACHE["nc"]

    ag, pg, ng = _host_gather(np.asarray(anchor), np.asarray(positive),
                              np.asarray(negative))
    in_maps = []
    for k in range(NCORES):
        m = {}
        for t in range(NTILES):
            sl = slice(UPC * k + 128 * t, UPC * k + 128 * t + 128)
            m[f"a{t}"] = ag[sl]
            m[f"p{t}"] = pg[sl]
            m[f"n{t}"] = ng[sl]
        in_maps.append(m)

    res = run_bass_kernel_spmd(nc, in_maps, list(range(NCORES)))
    _CACHE["last_results"] = res

    out_p = _assemble([res.results[k]["out_p"] for k in range(NCORES)])
    out_n = _assemble([res.results[k]["out_n"] for k in range(NCORES)])
    return out_p, out_n
